# revision 5
# baseline (speedup 1.0000x reference)
"""Trainium2 Bass kernel for the AdaptiveIzhikevichNeuron problem.

Reference: T=32 scan over 1M independent neurons (dt=1):
    v1 = 0.04 v^2 + 6 v + 140 - u + x_t ; spike = v1 >= 30
    v' = spike ? c : v1 ; u' = (1-a)u + ab*v1 + d*spike

Device formulation. Under the host-checked guards (every neuron spikes
at t=0 since min x_0 > -100, and v1c_1 < -1 everywhere), no neuron
spikes again for this input class, so the u-recurrence is linear and
the only nonlinearity is the square in the v-path. In exponentially
rescaled coordinates Vc_k = g_k * v1c_{t=k+2} with g_k = (1-a)^-k the
u-accumulation becomes a scale-free cumulative sum held in PSUM:

    s'_k  = Square(sc_k * Vc_{k-1} + bi_k)       [ScalarE]
    pre_k = x'_k + s'_k                          [DVE TT]
    Vc_k  = pre_k - P_k                          [DVE TT, PSUM operand]
    P_{k+1} = P_k + (ab/(1-a)) * Vc_k            [PE, constant diagonal]

P_0 = u_2 is seeded by one identity matmul from a host-computed row;
all per-step scales/constants (g_k, mu_k, E, beta) fold into the x'
rows (host), the activation scale/bias immediates, and the seed row.
No exit copy, no min op, no q join: the per-step device work is one
activation, two tensor-tensor ops, one matmul per half-block.

Output: Vc_k bf16 rows; spike <=> Vc_k >= 0 (sign margin |v1c| > 11
verified: 0/33.5M mismatches vs the f32 reference). Rows t=0,1 are
host-written (all-spike row and v1c_1 >= 0). If the guards fail or any
device-step spike appears (linear-dynamics assumption violated), the
host recomputes the exact reference in numpy instead.

Layout: host transposes x to time-major [T, M]; data parallel over 8
cores, core i owns neurons [i*131072, (i+1)*131072) as [128, 1024].
Input slots: row 0 = u_2 seed, row 1 = v1c_1 (initial Vc), rows 2..31
= x'_0..x'_29.
"""

import sys
from contextlib import ExitStack

import numpy as np

sys.path.insert(0, "/opt/trn_rl_repo")

import ml_dtypes  # noqa: E402

B, C, N, T = 16, 64, 1024, 32
M = B * C * N
N_CORES = 8
MC = M // N_CORES          # neurons per core
P = 128                    # SBUF partitions
F = MC // P                # free-dim elements per partition (1024)
H = F // 2                 # half-block width
K = T - 2                  # device steps

_CACHE: dict = {}


def _scalars(a, b, c, d):
    f32 = np.float32
    a, b, c, d = f32(a), f32(b), f32(c), f32(d)
    ab = a * b
    g = ((1.0 / (1.0 - np.float64(a))) ** np.arange(K)).astype(np.float32)
    mu = np.zeros(K, dtype=np.float32)
    for k in range(1, K):
        mu[k] = (1 - a) * mu[k - 1] + a * b * c
    E = -(c + 85)
    beta = c + 75
    h = ab / (1 - a)
    gprev = np.concatenate([[f32(1.0)], g[:-1]])
    sc = (f32(0.2) * np.sqrt(g) / gprev).astype(np.float32)
    bi = (f32(0.2) * np.sqrt(g) * beta).astype(np.float32)
    return ab, g, mu, E, beta, h, sc, bi


def _build(a: float, b: float, c: float, d: float):
    import concourse.bacc as bacc
    import concourse.tile as tile
    from concourse import mybir

    nc = bacc.Bacc("TRN2", target_bir_lowering=False, debug=False,
                   num_devices=N_CORES)
    bf16 = mybir.dt.bfloat16
    f32d = mybir.dt.float32
    Op = mybir.AluOpType
    Sq = mybir.ActivationFunctionType.Square

    x_ap = nc.dram_tensor("x", [T, P, F], bf16, kind="ExternalInput").ap()
    w_ap = nc.dram_tensor("wst", [P, P], bf16, kind="ExternalInput").ap()
    b_ap = nc.dram_tensor("biasT", [P, T], f32d, kind="ExternalInput").ap()
    out_ap = nc.dram_tensor("out", [K, P, F], bf16, kind="ExternalOutput").ap()

    _, _, _, _, _, _, sc, _ = _scalars(a, b, c, d)

    NB = 2
    Cp = mybir.ActivationFunctionType.Copy
    with tile.TileContext(nc, pool_alloc_mode="queue") as tc, ExitStack() as ctx:
        xp = ctx.enter_context(tc.tile_pool(name="xp", bufs=6))
        sp = ctx.enter_context(tc.tile_pool(name="sp", bufs=3))
        yp = ctx.enter_context(tc.tile_pool(name="yp", bufs=3))
        vp = ctx.enter_context(tc.tile_pool(name="vp", bufs=6))
        zp = ctx.enter_context(tc.tile_pool(name="zp", bufs=3))
        wp = ctx.enter_context(tc.tile_pool(name="wp", bufs=1))
        ps = ctx.enter_context(tc.tile_pool(name="ps", bufs=2, space="PSUM"))

        SH = wp.tile([P, P], bf16, tag="sh")     # diag(ab/(1-a))
        biasT = wp.tile([P, T], f32d, tag="bias")
        vinit = wp.tile([P, F], bf16, tag="vinit")
        u2 = wp.tile([P, F], bf16, tag="u2")     # u_2 / h
        dum = wp.tile([P, 1], f32d, tag="dum")
        psum = [ps.tile([P, H], f32d, tag=f"pw{j}", name=f"pw{j}")
                for j in range(NB)]
        sls = [slice(j * H, (j + 1) * H) for j in range(NB)]

        # Act-table preload via a dummy Square so the 1.3us table load
        # overlaps the input DMAs instead of stalling the first real one.
        nc.vector.memset(dum[:], 0.0)
        nc.scalar.activation(dum[:], dum[:], Sq, bias=dum[:], scale=1.0)
        # Sync queue carries the chain-critical preloads in dependency
        # order; biasT + stationary ride the GPSIMD software-DGE queue.
        nc.gpsimd.dma_start(out=biasT[:], in_=b_ap)
        nc.gpsimd.dma_start(out=SH[:], in_=w_ap)
        nc.sync.dma_start(out=vinit[:], in_=x_ap[1])
        nc.sync.dma_start(out=u2[:], in_=x_ap[0])

        Vc = [vinit[:, sls[j]] for j in range(NB)]
        Z1 = None

        for k in range(K):
            xt = xp.tile([P, F], bf16, tag="x")
            nc.sync.dma_start(out=xt[:], in_=x_ap[k + 2])
            if k == 0:
                # P_0 = diag(h) @ (u_2 / h) = u_2
                for j in range(NB):
                    nc.tensor.matmul(psum[j][:], SH[:], u2[:, sls[j]],
                                     start=True, stop=True)
                z = zp.tile([P, H], bf16, tag="z1", name="z1")
                nc.scalar.activation(z[:], psum[1][:], Cp, bias=0.0,
                                     scale=1.0)
                Z1 = z

            ss = [None] * NB
            for j in range(NB):
                s = sp.tile([P, H], bf16, tag=f"s{j}", name=f"s{j}")
                nc.scalar.activation(s[:], Vc[j], Sq,
                                     bias=biasT[:, k:k + 1],
                                     scale=float(sc[k]))
                ss[j] = s
            # One full-row Vc tile per step (halves are slices) so the
            # output DMA is a single [P, F] transfer. Block 0 subtracts
            # PSUM directly (1x DVE read); block 1 subtracts the bf16
            # SBUF snapshot Z1 (2x DVE) that ScalarE copies out of PSUM
            # after each matmul — balancing DVE against ScalarE.
            vt = vp.tile([P, F], bf16, tag="vc", name="vc")
            vcs = [vt[:, sls[j]] for j in range(NB)]
            for j in range(NB):
                pre = yp.tile([P, H], bf16, tag=f"pre{j}", name=f"pre{j}")
                nc.vector.tensor_tensor(pre[:], xt[:, sls[j]], ss[j][:],
                                        op=Op.add)
                nc.vector.tensor_tensor(vcs[j], pre[:],
                                        psum[0][:] if j == 0 else Z1[:],
                                        op=Op.subtract)
                if k < K - 1:
                    nc.tensor.matmul(psum[j][:], SH[:], vcs[j],
                                     start=False, stop=True,
                                     skip_group_check=True)
            if k < K - 1:
                z = zp.tile([P, H], bf16, tag="z1", name="z1")
                nc.scalar.activation(z[:], psum[1][:], Cp, bias=0.0,
                                     scale=1.0)
                Z1 = z
                nc.sync.dma_start(out=out_ap[k], in_=vt[:])
            else:
                for j in range(NB):
                    nc.sync.dma_start(out=out_ap[k][:, sls[j]], in_=vcs[j])
            Vc = vcs

    if not nc.is_finalized():
        nc.finalize()
    return nc


def _get_nc(a, b, c, d):
    key = (round(a, 9), round(b, 9), round(c, 9), round(d, 9))
    if key not in _CACHE:
        _CACHE[key] = _build(a, b, c, d)
    return _CACHE[key]


def _host_reference(x, a, b, c, d):
    """Exact f32 reference recompute (fallback path)."""
    f32 = np.float32
    a, b, c, d = f32(a), f32(b), f32(c), f32(d)
    xt = np.moveaxis(x.astype(np.float32), -1, 0)  # [T, B, C, N]
    v = np.zeros(xt.shape[1:], dtype=np.float32)
    u = np.zeros_like(v)
    out = np.empty_like(xt)
    for t in range(xt.shape[0]):
        v1 = f32(0.04) * v * v + 6 * v + 140 - u + xt[t]
        u1 = u + a * (b * v1 - u)
        spike = (v1 >= f32(30.0)).astype(np.float32)
        v = v1 * (1 - spike) + c * spike
        u = u1 + d * spike
        out[t] = spike
    return np.moveaxis(out, 0, -1)


def kernel(x, a, b, c, d, _trace=False):
    from concourse.bass_utils import run_bass_kernel_spmd

    a, b, c, d = (float(np.asarray(v)) for v in (a, b, c, d))
    xin = np.asarray(x)
    in_dtype = xin.dtype
    f32 = np.float32
    bf16 = ml_dtypes.bfloat16

    xtm = np.ascontiguousarray(xin.reshape(M, T).astype(np.float32).T)
    t0_all_spike = bool(xtm[0].min() > -100.0)
    ab, g, mu, E, beta, h, sc, bi = _scalars(a, b, c, d)
    if t0_all_spike:
        u1 = f32(ab) * (f32(140.0) + xtm[0]) + f32(d)
        v1c1 = (xtm[1] + (f32(0.04) * f32(c) * f32(c) + 6 * f32(c) + 140)
                - u1 - f32(c))
        t1_neg = bool(v1c1.max() < -1.0)
    else:
        t1_neg = False
    if not (t0_all_spike and t1_neg):
        out = _host_reference(xin, a, b, c, d).astype(in_dtype, copy=False)
        return (out, None) if _trace else out

    u2 = (1 - f32(a)) * u1 + f32(ab) * (v1c1 + f32(c))
    dev_in = np.empty((T, M), dtype=np.float32)
    dev_in[0] = u2 / f32(h)       # seeded through the diag(h) stationary
    dev_in[1] = v1c1
    for k in range(K):
        dev_in[k + 2] = g[k] * (E + xtm[k + 2] - mu[k])
    dev_in = dev_in.astype(bf16)

    wst = (f32(h) * np.eye(P, dtype=np.float32)).astype(bf16)
    biasT = np.zeros((P, T), dtype=np.float32)
    biasT[:, :K] = bi[None, :]

    nc = _get_nc(a, b, c, d)
    in_maps = [
        {"x": np.ascontiguousarray(dev_in[:, i * MC:(i + 1) * MC]
                                   ).reshape(T, P, F),
         "wst": wst, "biasT": biasT}
        for i in range(N_CORES)
    ]
    res = run_bass_kernel_spmd(nc, in_maps, core_ids=list(range(N_CORES)),
                               trace=_trace)
    rows = np.concatenate(
        [np.asarray(res.results[i]["out"]).reshape(K, MC)
         for i in range(N_CORES)],
        axis=1,
    )  # [K, M] of Vc_k bf16; spike <=> Vc >= 0
    spikes = np.zeros((T, M), dtype=np.float32)
    spikes[0] = 1.0
    spikes[1] = (v1c1 >= 0).astype(np.float32)
    spikes[2:] = (rows.astype(np.float32) >= 0).astype(np.float32)
    if spikes[1:].any():
        # a device-step spike violates the linear-dynamics assumption:
        # recompute exactly on host.
        out = _host_reference(xin, a, b, c, d).astype(in_dtype, copy=False)
        return (out, res) if _trace else out
    out = spikes.T.reshape(B, C, N, T).astype(in_dtype, copy=False)
    if _trace:
        return out, res
    return out


# revision 24
# speedup vs baseline: 1.1787x; 1.1787x over previous
"""Trainium2 Bass kernel for the AdaptiveIzhikevichNeuron problem.

Reference: T=32 scan over 1M independent neurons (dt=1):
    v1 = 0.04 v^2 + 6 v + 140 - u + x_t ; spike = v1 >= 30
    v' = spike ? c : v1 ; u' = (1-a)u + ab*v1 + d*spike

Device formulation. Under the host-checked guards (every neuron spikes
at t=0 since min x_0 > -100, and v1c_1 < -1 everywhere), no neuron
spikes again for this input class, so the u-recurrence is linear and
the only nonlinearity is the square in the v-path. In exponentially
rescaled coordinates Vc_k = g_k * v1c_{t=k+2} with g_k = (1-a)^-k the
u-accumulation becomes a scale-free cumulative sum held in PSUM:

    s'_k  = Square(sc_k * Vc_{k-1} + bi_k)       [ScalarE]
    pre_k = x'_k + s'_k                          [DVE TT, 2x mode]
    Vc_k  = pre_k - P_k                          [DVE TT]
    P_{k+1} = P_k + (ab/(1-a)) * Vc_k            [PE, constant diagonal
                                                  stationary: no per-step
                                                  ldweights stalls]

P_0 = u_2 is seeded through the same diag(h) stationary from a host
row carrying u_2/h; all per-step scales/constants (g_k, mu_k, E, beta)
fold into the x' rows (host), the activation scale immediates, and a
preloaded [P,32] bias table. No min op, no q join. Two half-blocks
pipeline the serial chain; block 0's Vc subtracts PSUM directly (1x
DVE read), block 1 subtracts a bf16 SBUF snapshot that ScalarE copies
out of PSUM each step (2x DVE read) — balancing DVE against ScalarE so
both run ~2.1us/step. PSUM stays f32, so the snapshot rounding does
not accumulate. Measured walls per step: ScalarE 2.13us = DVE-chain
2.12us >= DVE 1.95us; GPSIMD is left idle deliberately (its SWDGE DMAs
and Q7 compute contend for SBUF ports and throttle every engine ~20%).

Output: Vc_k bf16 rows; spike <=> Vc_k >= 0 (sign margin |v1c| > 11
verified: 0/33.5M mismatches vs the f32 reference). Rows t=0,1 are
host-written (all-spike row and v1c_1 >= 0). If the guards fail or any
device-step spike appears (linear-dynamics assumption violated), the
host recomputes the exact reference in numpy instead.

Layout: host transposes x to time-major [T, M]; data parallel over 8
cores, core i owns neurons [i*131072, (i+1)*131072) as [128, 1024].
Input slots: row 0 = u_2 seed, row 1 = v1c_1 (initial Vc), rows 2..31
= x'_0..x'_29.
"""

import sys
from contextlib import ExitStack

import numpy as np

sys.path.insert(0, "/opt/trn_rl_repo")

import ml_dtypes  # noqa: E402

B, C, N, T = 16, 64, 1024, 32
M = B * C * N
N_CORES = 8
MC = M // N_CORES          # neurons per core
P = 128                    # SBUF partitions
F = MC // P                # free-dim elements per partition (1024)
H = F // 2                 # half-block width
K = T - 2                  # device steps

_CACHE: dict = {}


def _scalars(a, b, c, d):
    f32 = np.float32
    a, b, c, d = f32(a), f32(b), f32(c), f32(d)
    ab = a * b
    g = ((1.0 / (1.0 - np.float64(a))) ** np.arange(K)).astype(np.float32)
    mu = np.zeros(K, dtype=np.float32)
    for k in range(1, K):
        mu[k] = (1 - a) * mu[k - 1] + a * b * c
    E = -(c + 85)
    beta = c + 75
    h = ab / (1 - a)
    gprev = np.concatenate([[f32(1.0)], g[:-1]])
    sc = (f32(0.2) * np.sqrt(g) / gprev).astype(np.float32)
    bi = (f32(0.2) * np.sqrt(g) * beta).astype(np.float32)
    return ab, g, mu, E, beta, h, sc, bi


def _build(a: float, b: float, c: float, d: float):
    import concourse.bacc as bacc
    import concourse.tile as tile
    from concourse import mybir

    nc = bacc.Bacc("TRN2", target_bir_lowering=False, debug=False,
                   num_devices=N_CORES)
    bf16 = mybir.dt.bfloat16
    f32d = mybir.dt.float32
    Op = mybir.AluOpType
    Sq = mybir.ActivationFunctionType.Square

    x_ap = nc.dram_tensor("x", [T, P, F], bf16, kind="ExternalInput").ap()
    w_ap = nc.dram_tensor("aux", [P, F + P], bf16, kind="ExternalInput").ap()
    b_ap = nc.dram_tensor("biasT", [P, T], f32d, kind="ExternalInput").ap()
    out_ap = nc.dram_tensor("out", [K, P, F], bf16, kind="ExternalOutput").ap()

    _, _, _, _, _, _, sc, _ = _scalars(a, b, c, d)

    NB = 2
    Cp = mybir.ActivationFunctionType.Copy
    with tile.TileContext(nc, pool_alloc_mode="queue") as tc, ExitStack() as ctx:
        xp = ctx.enter_context(tc.tile_pool(name="xp", bufs=6))
        sp = ctx.enter_context(tc.tile_pool(name="sp", bufs=3))
        yp = ctx.enter_context(tc.tile_pool(name="yp", bufs=3))
        vp = ctx.enter_context(tc.tile_pool(name="vp", bufs=6))
        zp = ctx.enter_context(tc.tile_pool(name="zp", bufs=3))
        wp = ctx.enter_context(tc.tile_pool(name="wp", bufs=1))
        ps = ctx.enter_context(tc.tile_pool(name="ps", bufs=2, space="PSUM"))

        aux = wp.tile([P, F + P], bf16, tag="aux")  # [u_2/h | diag(ab/(1-a))]
        u2 = aux[:, :F]
        SH = aux[:, F:F + P]
        biasT = wp.tile([P, T], f32d, tag="bias")
        vinit = wp.tile([P, F], bf16, tag="vinit")
        dum = wp.tile([P, 1], f32d, tag="dum")
        psum = [ps.tile([P, H], f32d, tag=f"pw{j}", name=f"pw{j}")
                for j in range(NB)]
        sls = [slice(j * H, (j + 1) * H) for j in range(NB)]

        # Act-table preload via a dummy Square so the 1.3us table load
        # overlaps the input DMAs instead of stalling the first real one.
        nc.vector.memset(dum[:], 0.0)
        nc.scalar.activation(dum[:], dum[:], Sq, bias=dum[:], scale=1.0)
        # Preloads: biasT rides the idle Activation HWDGE queue so the
        # Sync queue's vinit/aux completion-sem latencies overlap it.
        nc.scalar.dma_start(out=biasT[:], in_=b_ap)
        nc.sync.dma_start(out=vinit[:], in_=x_ap[1])
        nc.sync.dma_start(out=aux[:], in_=w_ap)

        Vc = [vinit[:, sls[j]] for j in range(NB)]
        Z1 = None

        for k in range(K):
            xt = xp.tile([P, F], bf16, tag="x")
            nc.sync.dma_start(out=xt[:], in_=x_ap[k + 2])
            if k == 0:
                # P_0 = diag(h) @ (u_2 / h) = u_2
                for j in range(NB):
                    nc.tensor.matmul(psum[j][:], SH, u2[:, sls[j]],
                                     start=True, stop=True)

            ss = [None] * NB
            for j in range(NB):
                s = sp.tile([P, H], bf16, tag=f"s{j}", name=f"s{j}")
                nc.scalar.activation(s[:], Vc[j], Sq,
                                     bias=biasT[:, k:k + 1],
                                     scale=float(sc[k]))
                ss[j] = s
            if k == 0:
                # seed snapshot emitted after the Squares — it depends on
                # the seed matmul and must not head-of-line-block them.
                z = zp.tile([P, H], bf16, tag="z1", name="z1")
                nc.scalar.activation(z[:], psum[1][:], Cp, bias=0.0,
                                     scale=1.0)
                Z1 = z
            # One full-row Vc tile per step (halves are slices) so the
            # output DMA is a single [P, F] transfer. Block 0 subtracts
            # PSUM directly (1x DVE read); block 1 subtracts the bf16
            # SBUF snapshot Z1 (2x DVE) that ScalarE copies out of PSUM
            # after each matmul — balancing DVE against ScalarE.
            vt = vp.tile([P, F], bf16, tag="vc", name="vc")
            vcs = [vt[:, sls[j]] for j in range(NB)]
            for j in range(NB):
                pre = yp.tile([P, H], bf16, tag=f"pre{j}", name=f"pre{j}")
                nc.vector.tensor_tensor(pre[:], xt[:, sls[j]], ss[j][:],
                                        op=Op.add)
                nc.vector.tensor_tensor(vcs[j], pre[:],
                                        psum[0][:] if j == 0 else Z1[:],
                                        op=Op.subtract)
                if k < K - 1:
                    nc.tensor.matmul(psum[j][:], SH, vcs[j],
                                     start=False, stop=True,
                                     skip_group_check=True)
            if k < K - 1:
                z = zp.tile([P, H], bf16, tag="z1", name="z1")
                nc.scalar.activation(z[:], psum[1][:], Cp, bias=0.0,
                                     scale=1.0)
                Z1 = z
                nc.sync.dma_start(out=out_ap[k], in_=vt[:])
            else:
                for j in range(NB):
                    nc.sync.dma_start(out=out_ap[k][:, sls[j]], in_=vcs[j])
            Vc = vcs

    if not nc.is_finalized():
        nc.finalize()
    return nc


def _get_nc(a, b, c, d):
    key = (round(a, 9), round(b, 9), round(c, 9), round(d, 9))
    if key not in _CACHE:
        _CACHE[key] = _build(a, b, c, d)
    return _CACHE[key]


def _host_reference(x, a, b, c, d):
    """Exact f32 reference recompute (fallback path)."""
    f32 = np.float32
    a, b, c, d = f32(a), f32(b), f32(c), f32(d)
    xt = np.moveaxis(x.astype(np.float32), -1, 0)  # [T, B, C, N]
    v = np.zeros(xt.shape[1:], dtype=np.float32)
    u = np.zeros_like(v)
    out = np.empty_like(xt)
    for t in range(xt.shape[0]):
        v1 = f32(0.04) * v * v + 6 * v + 140 - u + xt[t]
        u1 = u + a * (b * v1 - u)
        spike = (v1 >= f32(30.0)).astype(np.float32)
        v = v1 * (1 - spike) + c * spike
        u = u1 + d * spike
        out[t] = spike
    return np.moveaxis(out, 0, -1)


def kernel(x, a, b, c, d, _trace=False):
    from concourse.bass_utils import run_bass_kernel_spmd

    a, b, c, d = (float(np.asarray(v)) for v in (a, b, c, d))
    xin = np.asarray(x)
    in_dtype = xin.dtype
    f32 = np.float32
    bf16 = ml_dtypes.bfloat16

    xtm = np.ascontiguousarray(xin.reshape(M, T).astype(np.float32).T)
    t0_all_spike = bool(xtm[0].min() > -100.0)
    ab, g, mu, E, beta, h, sc, bi = _scalars(a, b, c, d)
    if t0_all_spike:
        u1 = f32(ab) * (f32(140.0) + xtm[0]) + f32(d)
        v1c1 = (xtm[1] + (f32(0.04) * f32(c) * f32(c) + 6 * f32(c) + 140)
                - u1 - f32(c))
        t1_neg = bool(v1c1.max() < -1.0)
    else:
        t1_neg = False
    if not (t0_all_spike and t1_neg):
        out = _host_reference(xin, a, b, c, d).astype(in_dtype, copy=False)
        return (out, None) if _trace else out

    u2 = (1 - f32(a)) * u1 + f32(ab) * (v1c1 + f32(c))
    dev_in = np.empty((T, M), dtype=np.float32)
    dev_in[0] = u2 / f32(h)       # seeded through the diag(h) stationary
    dev_in[1] = v1c1
    for k in range(K):
        dev_in[k + 2] = g[k] * (E + xtm[k + 2] - mu[k])
    dev_in = dev_in.astype(bf16)

    sh = (f32(h) * np.eye(P, dtype=np.float32)).astype(bf16)
    biasT = np.zeros((P, T), dtype=np.float32)
    biasT[:, :K] = bi[None, :]

    nc = _get_nc(a, b, c, d)
    in_maps = [
        {"x": np.ascontiguousarray(dev_in[:, i * MC:(i + 1) * MC]
                                   ).reshape(T, P, F),
         "aux": np.ascontiguousarray(np.concatenate(
             [dev_in[0, i * MC:(i + 1) * MC].reshape(P, F), sh], axis=1)),
         "biasT": biasT}
        for i in range(N_CORES)
    ]
    res = run_bass_kernel_spmd(nc, in_maps, core_ids=list(range(N_CORES)),
                               trace=_trace)
    rows = np.concatenate(
        [np.asarray(res.results[i]["out"]).reshape(K, MC)
         for i in range(N_CORES)],
        axis=1,
    )  # [K, M] of Vc_k bf16; spike <=> Vc >= 0
    spikes = np.zeros((T, M), dtype=np.float32)
    spikes[0] = 1.0
    spikes[1] = (v1c1 >= 0).astype(np.float32)
    spikes[2:] = (rows.astype(np.float32) >= 0).astype(np.float32)
    if spikes[1:].any():
        # a device-step spike violates the linear-dynamics assumption:
        # recompute exactly on host.
        out = _host_reference(xin, a, b, c, d).astype(in_dtype, copy=False)
        return (out, res) if _trace else out
    out = spikes.T.reshape(B, C, N, T).astype(in_dtype, copy=False)
    if _trace:
        return out, res
    return out


# revision 25
# speedup vs baseline: 1.1891x; 1.0088x over previous
"""Trainium2 Bass kernel for the AdaptiveIzhikevichNeuron problem.

Reference: T=32 scan over 1M independent neurons (dt=1):
    v1 = 0.04 v^2 + 6 v + 140 - u + x_t ; spike = v1 >= 30
    v' = spike ? c : v1 ; u' = (1-a)u + ab*v1 + d*spike

Device formulation. Under the host-checked guards (every neuron spikes
at t=0 since min x_0 > -100, and v1c_1 < -1 everywhere), no neuron
spikes again for this input class, so the u-recurrence is linear and
the only nonlinearity is the square in the v-path. In exponentially
rescaled coordinates Vc_k = g_k * v1c_{t=k+2} with g_k = (1-a)^-k the
u-accumulation becomes a scale-free cumulative sum held in PSUM:

    s'_k  = Square(sc_k * Vc_{k-1} + bi_k)       [ScalarE]
    pre_k = x'_k + s'_k                          [DVE TT, 2x mode]
    Vc_k  = pre_k - P_k                          [DVE TT]
    P_{k+1} = P_k + (ab/(1-a)) * Vc_k            [PE, constant diagonal
                                                  stationary: no per-step
                                                  ldweights stalls]

P_0 = u_2 is seeded through the same diag(h) stationary from a host
row carrying u_2/h; all per-step scales/constants (g_k, mu_k, E, beta)
fold into the x' rows (host), the activation scale immediates, and a
preloaded [P,32] bias table. No min op, no q join. Two half-blocks
pipeline the serial chain; block 0's Vc subtracts PSUM directly (1x
DVE read), block 1 subtracts a bf16 SBUF snapshot that ScalarE copies
out of PSUM each step (2x DVE read) — balancing DVE against ScalarE so
both run ~2.1us/step. PSUM stays f32, so the snapshot rounding does
not accumulate. Measured walls per step: ScalarE 2.13us = DVE-chain
2.12us >= DVE 1.95us; GPSIMD is left idle deliberately (its SWDGE DMAs
and Q7 compute contend for SBUF ports and throttle every engine ~20%).

Output: Vc_k bf16 rows; spike <=> Vc_k >= 0 (sign margin |v1c| > 11
verified: 0/33.5M mismatches vs the f32 reference). Rows t=0,1 are
host-written (all-spike row and v1c_1 >= 0). If the guards fail or any
device-step spike appears (linear-dynamics assumption violated), the
host recomputes the exact reference in numpy instead.

Layout: host transposes x to time-major [T, M]; data parallel over 8
cores, core i owns neurons [i*131072, (i+1)*131072) as [128, 1024].
Input slots: row 1 = v1c_1 (initial Vc), rows 2..31 = x'_0..x'_29;
the u_2/h seed row and the diag(h) stationary arrive packed in one
"aux" tensor (row 0 of x is unused on device).
"""

import sys
from contextlib import ExitStack

import numpy as np

sys.path.insert(0, "/opt/trn_rl_repo")

import ml_dtypes  # noqa: E402

B, C, N, T = 16, 64, 1024, 32
M = B * C * N
N_CORES = 8
MC = M // N_CORES          # neurons per core
P = 128                    # SBUF partitions
F = MC // P                # free-dim elements per partition (1024)
H = F // 2                 # half-block width
K = T - 2                  # device steps

_CACHE: dict = {}


def _scalars(a, b, c, d):
    f32 = np.float32
    a, b, c, d = f32(a), f32(b), f32(c), f32(d)
    ab = a * b
    g = ((1.0 / (1.0 - np.float64(a))) ** np.arange(K)).astype(np.float32)
    mu = np.zeros(K, dtype=np.float32)
    for k in range(1, K):
        mu[k] = (1 - a) * mu[k - 1] + a * b * c
    E = -(c + 85)
    beta = c + 75
    h = ab / (1 - a)
    gprev = np.concatenate([[f32(1.0)], g[:-1]])
    sc = (f32(0.2) * np.sqrt(g) / gprev).astype(np.float32)
    bi = (f32(0.2) * np.sqrt(g) * beta).astype(np.float32)
    return ab, g, mu, E, beta, h, sc, bi


def _build(a: float, b: float, c: float, d: float):
    import concourse.bacc as bacc
    import concourse.tile as tile
    from concourse import mybir

    nc = bacc.Bacc("TRN2", target_bir_lowering=False, debug=False,
                   num_devices=N_CORES)
    bf16 = mybir.dt.bfloat16
    f32d = mybir.dt.float32
    Op = mybir.AluOpType
    Sq = mybir.ActivationFunctionType.Square

    x_ap = nc.dram_tensor("x", [T, P, F], bf16, kind="ExternalInput").ap()
    w_ap = nc.dram_tensor("aux", [P, F + P], bf16, kind="ExternalInput").ap()
    b_ap = nc.dram_tensor("biasT", [P, T], f32d, kind="ExternalInput").ap()
    out_ap = nc.dram_tensor("out", [K, P, F], bf16, kind="ExternalOutput").ap()

    _, _, _, _, _, _, sc, _ = _scalars(a, b, c, d)

    NB = 2
    Cp = mybir.ActivationFunctionType.Copy
    with tile.TileContext(nc, pool_alloc_mode="queue") as tc, ExitStack() as ctx:
        xp = ctx.enter_context(tc.tile_pool(name="xp", bufs=6))
        sp = ctx.enter_context(tc.tile_pool(name="sp", bufs=3))
        yp = ctx.enter_context(tc.tile_pool(name="yp", bufs=3))
        vp = ctx.enter_context(tc.tile_pool(name="vp", bufs=6))
        zp = ctx.enter_context(tc.tile_pool(name="zp", bufs=3))
        wp = ctx.enter_context(tc.tile_pool(name="wp", bufs=1))
        ps = ctx.enter_context(tc.tile_pool(name="ps", bufs=2, space="PSUM"))

        aux = wp.tile([P, F + P], bf16, tag="aux")  # [u_2/h | diag(ab/(1-a))]
        u2 = aux[:, :F]
        SH = aux[:, F:F + P]
        biasT = wp.tile([P, T], f32d, tag="bias")
        vinit = wp.tile([P, F], bf16, tag="vinit")
        dum = wp.tile([P, 1], f32d, tag="dum")
        psum = [ps.tile([P, H], f32d, tag=f"pw{j}", name=f"pw{j}")
                for j in range(NB)]
        sls = [slice(j * H, (j + 1) * H) for j in range(NB)]

        # Act-table preload via a dummy Square so the 1.3us table load
        # overlaps the input DMAs instead of stalling the first real one.
        nc.vector.memset(dum[:], 0.0)
        nc.scalar.activation(dum[:], dum[:], Sq, bias=dum[:], scale=1.0)
        # Preloads: biasT rides the idle Activation HWDGE queue so the
        # Sync queue's vinit/aux completion-sem latencies overlap it.
        nc.scalar.dma_start(out=biasT[:], in_=b_ap)
        nc.sync.dma_start(out=vinit[:], in_=x_ap[1])
        nc.sync.dma_start(out=aux[:], in_=w_ap)

        Vc = [vinit[:, sls[j]] for j in range(NB)]
        Z1 = None

        for k in range(K):
            xt = xp.tile([P, F], bf16, tag="x")
            nc.sync.dma_start(out=xt[:], in_=x_ap[k + 2])
            if k == 0:
                # P_0 = diag(h) @ (u_2 / h) = u_2
                for j in range(NB):
                    nc.tensor.matmul(psum[j][:], SH, u2[:, sls[j]],
                                     start=True, stop=True)

            ss = [None] * NB
            for j in range(NB):
                s = sp.tile([P, H], bf16, tag=f"s{j}", name=f"s{j}")
                nc.scalar.activation(s[:], Vc[j], Sq,
                                     bias=biasT[:, k:k + 1],
                                     scale=float(sc[k]))
                ss[j] = s
            if k == 0:
                # seed snapshot emitted after the Squares — it depends on
                # the seed matmul and must not head-of-line-block them.
                z = zp.tile([P, H], bf16, tag="z1", name="z1")
                nc.scalar.activation(z[:], psum[1][:], Cp, bias=0.0,
                                     scale=1.0)
                Z1 = z
            # One full-row Vc tile per step (halves are slices) so the
            # output DMA is a single [P, F] transfer. Block 0 subtracts
            # PSUM directly (1x DVE read); block 1 subtracts the bf16
            # SBUF snapshot Z1 (2x DVE) that ScalarE copies out of PSUM
            # after each matmul — balancing DVE against ScalarE.
            vt = vp.tile([P, F], bf16, tag="vc", name="vc")
            vcs = [vt[:, sls[j]] for j in range(NB)]
            for j in range(NB):
                pre = yp.tile([P, H], bf16, tag=f"pre{j}", name=f"pre{j}")
                nc.vector.tensor_tensor(pre[:], xt[:, sls[j]], ss[j][:],
                                        op=Op.add)
                nc.vector.tensor_tensor(vcs[j], pre[:],
                                        psum[0][:] if j == 0 else Z1[:],
                                        op=Op.subtract)
                if k < K - 1:
                    nc.tensor.matmul(psum[j][:], SH, vcs[j],
                                     start=False, stop=True,
                                     skip_group_check=True)
            if k < K - 1:
                z = zp.tile([P, H], bf16, tag="z1", name="z1")
                nc.scalar.activation(z[:], psum[1][:], Cp, bias=0.0,
                                     scale=1.0)
                Z1 = z
                nc.sync.dma_start(out=out_ap[k], in_=vt[:])
            else:
                for j in range(NB):
                    nc.sync.dma_start(out=out_ap[k][:, sls[j]], in_=vcs[j])
            Vc = vcs

    if not nc.is_finalized():
        nc.finalize()
    return nc


def _get_nc(a, b, c, d):
    key = (round(a, 9), round(b, 9), round(c, 9), round(d, 9))
    if key not in _CACHE:
        _CACHE[key] = _build(a, b, c, d)
    return _CACHE[key]


def _host_reference(x, a, b, c, d):
    """Exact f32 reference recompute (fallback path)."""
    f32 = np.float32
    a, b, c, d = f32(a), f32(b), f32(c), f32(d)
    xt = np.moveaxis(x.astype(np.float32), -1, 0)  # [T, B, C, N]
    v = np.zeros(xt.shape[1:], dtype=np.float32)
    u = np.zeros_like(v)
    out = np.empty_like(xt)
    for t in range(xt.shape[0]):
        v1 = f32(0.04) * v * v + 6 * v + 140 - u + xt[t]
        u1 = u + a * (b * v1 - u)
        spike = (v1 >= f32(30.0)).astype(np.float32)
        v = v1 * (1 - spike) + c * spike
        u = u1 + d * spike
        out[t] = spike
    return np.moveaxis(out, 0, -1)


def kernel(x, a, b, c, d, _trace=False):
    from concourse.bass_utils import run_bass_kernel_spmd

    a, b, c, d = (float(np.asarray(v)) for v in (a, b, c, d))
    xin = np.asarray(x)
    in_dtype = xin.dtype
    f32 = np.float32
    bf16 = ml_dtypes.bfloat16

    xtm = np.ascontiguousarray(xin.reshape(M, T).astype(np.float32).T)
    t0_all_spike = bool(xtm[0].min() > -100.0)
    ab, g, mu, E, beta, h, sc, bi = _scalars(a, b, c, d)
    if t0_all_spike:
        u1 = f32(ab) * (f32(140.0) + xtm[0]) + f32(d)
        v1c1 = (xtm[1] + (f32(0.04) * f32(c) * f32(c) + 6 * f32(c) + 140)
                - u1 - f32(c))
        t1_neg = bool(v1c1.max() < -1.0)
    else:
        t1_neg = False
    if not (t0_all_spike and t1_neg):
        out = _host_reference(xin, a, b, c, d).astype(in_dtype, copy=False)
        return (out, None) if _trace else out

    u2 = (1 - f32(a)) * u1 + f32(ab) * (v1c1 + f32(c))
    dev_in = np.empty((T, M), dtype=np.float32)
    dev_in[0] = u2 / f32(h)       # seeded through the diag(h) stationary
    dev_in[1] = v1c1
    for k in range(K):
        dev_in[k + 2] = g[k] * (E + xtm[k + 2] - mu[k])
    dev_in = dev_in.astype(bf16)

    sh = (f32(h) * np.eye(P, dtype=np.float32)).astype(bf16)
    biasT = np.zeros((P, T), dtype=np.float32)
    biasT[:, :K] = bi[None, :]

    nc = _get_nc(a, b, c, d)
    in_maps = [
        {"x": np.ascontiguousarray(dev_in[:, i * MC:(i + 1) * MC]
                                   ).reshape(T, P, F),
         "aux": np.ascontiguousarray(np.concatenate(
             [dev_in[0, i * MC:(i + 1) * MC].reshape(P, F), sh], axis=1)),
         "biasT": biasT}
        for i in range(N_CORES)
    ]
    res = run_bass_kernel_spmd(nc, in_maps, core_ids=list(range(N_CORES)),
                               trace=_trace)
    rows = np.concatenate(
        [np.asarray(res.results[i]["out"]).reshape(K, MC)
         for i in range(N_CORES)],
        axis=1,
    )  # [K, M] of Vc_k bf16; spike <=> Vc >= 0
    spikes = np.zeros((T, M), dtype=np.float32)
    spikes[0] = 1.0
    spikes[1] = (v1c1 >= 0).astype(np.float32)
    spikes[2:] = (rows.astype(np.float32) >= 0).astype(np.float32)
    if spikes[1:].any():
        # a device-step spike violates the linear-dynamics assumption:
        # recompute exactly on host.
        out = _host_reference(xin, a, b, c, d).astype(in_dtype, copy=False)
        return (out, res) if _trace else out
    out = spikes.T.reshape(B, C, N, T).astype(in_dtype, copy=False)
    if _trace:
        return out, res
    return out
